# revision 7
# baseline (speedup 1.0000x reference)
"""Trainium2 Bass kernel for nn_AdaptivePrototypeRefiner.

8-core SPMD, data-parallel over the N_q axis (2048 queries/core).

Math (per reference):
  refinement loop (3 steps): soft = softmax(-d/T), wmean = soft.T@qf / clamp(soft.sum(0)),
  refined += 0.1 * MLP([refined, wmean]).  The soft/wmean part is independent of
  `refined`, so all 3 steps' weighted sums are computed locally, AllReduced once,
  and the tiny MLP chain runs replicated on every core.
  confidence: conf[k] = mean_n sigmoid(relu(hp[k]+hq[n]+bc1) @ Wc2 + bc2);
  per-core partial sums of tanh are AllReduced (sigmoid(z)=0.5+0.5*tanh(z/2)).
"""

import sys

for _p in ("/opt/trn_rl_repo",):
    if _p not in sys.path:
        sys.path.append(_p)

import numpy as np
import ml_dtypes

import concourse.bass as bass
import concourse.bacc as bacc
import concourse.mybir as mybir
import concourse.tile as tile
from concourse.bass_utils import run_bass_kernel_spmd

F32 = mybir.dt.float32
BF16 = mybir.dt.bfloat16
BF = ml_dtypes.bfloat16

N_CORES = 8
K = 32          # classes
NQ = 16384      # queries total
C = 512         # feature dim
HR = 512        # refinement hidden
HC = 256        # confidence hidden
S = 3           # refinement steps
NQS = NQ // N_CORES   # 2048 queries per core
NB = NQS // 128       # 16 blocks of 128 queries
NCH = NQS // 512      # 4 chunks of 512 queries (psum-bank sized)

TRACE = False           # set by test harness for profiling runs
TRACE_KWARGS = {}

_CACHE = {}

AF = mybir.ActivationFunctionType
OP = mybir.AluOpType


def _build():
    nc = bacc.Bacc("TRN2", target_bir_lowering=False, debug=False,
                   num_devices=N_CORES)

    # ---------------- dram parameters (per-core shards / replicated) ---------
    P = lambda name, shape, dt: nc.declare_dram_parameter(name, list(shape), dt, isOutput=False)
    qfn_d = P("qfn", (128, NB, C), BF16)          # qf shard, n = p*16+g
    qft_d = P("qft", (128, C // 128, NQS), BF16)  # qf shard transposed [c%128, c//128, (g,p)]
    dist_d = P("dist", (128, NB, K), F32)         # distances shard
    w1_d = P("w1", (128, 8, HR), BF16)            # W1 [cc%128, cc//128, h]
    w2_d = P("w2", (128, 4, C), BF16)             # W2 [h%128, h//128, c]
    wq_d = P("wq", (128, 4, HC), BF16)            # Wc1[512:] blocks
    wp_d = P("wp", (128, 4, HC), BF16)            # Wc1[:512] blocks
    w2d_d = P("w2d", (128, 2, K, K), BF16)        # diag-expanded Wc2
    ptT_d = P("ptT", (128, 4, K), BF16)           # prototypes^T blocks
    pt_d = P("pt", (K, C), F32)                   # prototypes (refined init)
    b1_d = P("b1r", (1, HR), BF16)
    b2_d = P("b2r", (1, C), BF16)
    bc1_d = P("bc1r", (1, HC), BF16)
    bc2_d = P("bc2c", (128, 1), F32)              # bc2 replicated down partitions
    id_d = P("ident", (128, 128), BF16)

    ref_out = nc.declare_dram_parameter("refined", [K, C], F32, isOutput=True)
    conf_out = nc.declare_dram_parameter("conf", [K, 1], F32, isOutput=True)

    with tile.TileContext(nc) as tc:
        with (
            tc.tile_pool(name="sbuf", bufs=1) as sb,
            tc.tile_pool(name="rpool", bufs=6) as rp,
            tc.tile_pool(name="psA", bufs=4, space="PSUM") as psA,
            tc.tile_pool(name="psH", bufs=2, space="PSUM") as psH,
            tc.tile_pool(name="psG", bufs=2, space="PSUM") as psG,
            tc.tile_pool(name="dram", bufs=1, space="DRAM") as dram,
        ):
            # PSUM budget (8 banks): conf_ps 4 + hq_ps 2 + ps_gen 2
            gen_tile = lambda shape, dt: psG.tile(list(shape), dt, tag="ps_gen",
                                                  name="ps_gen")
            # ------------- input DMA to SBUF ---------------------------------
            def load(dparam, shape, dt, name):
                t = sb.tile(list(shape), dt, tag=name)
                nc.sync.dma_start(out=t[:], in_=dparam[:])
                return t

            qfn = load(qfn_d, (128, NB, C), BF16, "qfn")
            qft = load(qft_d, (128, 4, NQS), BF16, "qft")
            dist = load(dist_d, (128, NB, K), F32, "dist")
            w1 = load(w1_d, (128, 8, HR), BF16, "w1")
            w2 = load(w2_d, (128, 4, C), BF16, "w2")
            wq = load(wq_d, (128, 4, HC), BF16, "wq")
            wp = load(wp_d, (128, 4, HC), BF16, "wp")
            w2d = load(w2d_d, (128, 2, K, K), BF16, "w2d")
            ptT = load(ptT_d, (128, 4, K), BF16, "ptT")
            b1r = load(b1_d, (1, HR), BF16, "b1r")
            b2r = load(b2_d, (1, C), BF16, "b2r")
            bc1r = load(bc1_d, (1, HC), BF16, "bc1r")
            bc2c = load(bc2_d, (128, 1), F32, "bc2c")
            ident = load(id_d, (128, 128), BF16, "ident")

            ones_row = sb.tile([1, K], BF16, tag="ones_row")
            nc.vector.memset(ones_row[:], 1.0)
            ones_col = sb.tile([128, 1], BF16, tag="ones_col")
            nc.vector.memset(ones_col[:], 1.0)

            # ------------- stage A: softmax + weighted-sum partials ----------
            esb = sb.tile([128, NB, S, K], F32, tag="esb")
            for s in range(S):
                nc.scalar.activation(esb[:, :, s, :], dist[:], AF.Exp,
                                     scale=-1.0 / (s + 1.0))
            zsb = sb.tile([128, NB, S], F32, tag="zsb")
            nc.vector.tensor_reduce(zsb[:], esb[:], axis=mybir.AxisListType.X,
                                    op=OP.add)
            rz = sb.tile([128, NB, S], F32, tag="rz")
            nc.vector.reciprocal(rz[:], zsb[:])
            soft = sb.tile([128, NB, S, K], BF16, tag="soft")
            rz_ap = rz[:]
            rz_b = bass.AP(rz_ap.tensor, rz_ap.offset,
                           [rz_ap.ap[0], rz_ap.ap[1], rz_ap.ap[2], [0, K]])
            nc.vector.tensor_tensor(out=soft[:], in0=esb[:], in1=rz_b,
                                    op=OP.mult)

            num_ps = gen_tile([S * K, C], F32)
            wsum_ps = gen_tile([S * K, 1], F32)
            for g in range(NB):
                st, sp = (g == 0), (g == NB - 1)
                lhsT = soft[:, g]                # (128, S, K) -> 96 contiguous cols
                nc.tensor.matmul(num_ps[:], lhsT, qfn[:, g, :],
                                 start=st, stop=sp)
                nc.tensor.matmul(wsum_ps[:], lhsT, ones_col[:],
                                 start=st, stop=sp)
            num_sb = sb.tile([S * K, C], F32, tag="num_sb")
            wsum_sb = sb.tile([S * K, 1], F32, tag="wsum_sb")
            nc.vector.tensor_copy(num_sb[:], num_ps[:])
            nc.vector.tensor_copy(wsum_sb[:], wsum_ps[:])

            # ------------- AllReduce #1 (196KB) ------------------------------
            ar1_in = dram.tile([S * K, C + 1], F32)
            ar1_out = dram.tile([S * K, C + 1], F32)
            nc.gpsimd.dma_start(out=ar1_in[:, :C], in_=num_sb[:])
            nc.gpsimd.dma_start(out=ar1_in[:, C:], in_=wsum_sb[:])
            nc.gpsimd.collective_compute(
                "AllReduce", OP.add,
                replica_groups=[list(range(N_CORES))],
                ins=[ar1_in.opt()], outs=[ar1_out.opt()],
            )
            arn = sb.tile([S * K, C + 1], F32, tag="arn")
            nc.gpsimd.dma_start(out=arn[:], in_=ar1_out[:])

            # ------------- confidence setup: hp, hq --------------------------
            # hpT blocks (128h, K) = Wp_blk.T @ protoT_blk + bc1
            hpbT = sb.tile([128, 2, K], F32, tag="hpbT")
            for hb in range(2):
                hp_ps = gen_tile([128, K], F32)
                for cb in range(4):
                    nc.tensor.matmul(hp_ps[:],
                                     wp[:, cb, hb * 128:(hb + 1) * 128],
                                     ptT[:, cb, :],
                                     start=(cb == 0), stop=False)
                nc.tensor.matmul(hp_ps[:], bc1r[:, hb * 128:(hb + 1) * 128],
                                 ones_row[:], start=False, stop=True)
                nc.vector.tensor_copy(hpbT[:, hb, :], hp_ps[:])

            # hqT (h-part, n-free) bf16
            hqt = sb.tile([128, 2, NQS], BF16, tag="hqt")
            for hb in range(2):
                for j in range(NCH):
                    hq_ps = psH.tile([128, 512], F32, tag="hq_ps")
                    for cb in range(4):
                        nc.tensor.matmul(hq_ps[:],
                                         wq[:, cb, hb * 128:(hb + 1) * 128],
                                         qft[:, cb, j * 512:(j + 1) * 512],
                                         start=(cb == 0), stop=(cb == 3))
                    nc.vector.tensor_copy(hqt[:, hb, j * 512:(j + 1) * 512],
                                          hq_ps[:])

            # ------------- refinement MLP chain (after AR1) ------------------
            wsc = sb.tile([S * K, 1], F32, tag="wsc")
            nc.vector.tensor_scalar(out=wsc[:], in0=arn[:, C:], scalar1=1e-6,
                                    scalar2=None, op0=OP.max)
            rws = sb.tile([S * K, 1], F32, tag="rws")
            nc.vector.reciprocal(rws[:], wsc[:])
            wmean = sb.tile([S * K, C], BF16, tag="wmean")
            nc.vector.tensor_scalar(out=wmean[:], in0=arn[:, :C],
                                    scalar1=rws[:], scalar2=None, op0=OP.mult)

            ref_f = []
            for s in range(S + 1):
                ref_f.append(sb.tile([K, C], F32, tag=f"ref_f{s}", name=f"ref_f{s}"))
            nc.sync.dma_start(out=ref_f[0][:], in_=pt_d[:])
            ref_bf = sb.tile([K, C], BF16, tag="ref_bf")
            nc.vector.tensor_copy(ref_bf[:], ref_f[0][:])

            for s in range(S):
                # concat^T blocks: refined part (cb 0-3), wmean part (cb 4-7)
                catT = sb.tile([128, 8, K], BF16, tag="catT")
                for cb in range(4):
                    tr_ps = gen_tile([128, K], BF16)
                    nc.tensor.transpose(tr_ps[:],
                                        ref_bf[:, cb * 128:(cb + 1) * 128],
                                        ident[:K, :K])
                    nc.vector.tensor_copy(catT[:, cb, :], tr_ps[:])
                for cb in range(4):
                    tr_ps = gen_tile([128, K], BF16)
                    nc.tensor.transpose(
                        tr_ps[:],
                        wmean[s * K:(s + 1) * K, cb * 128:(cb + 1) * 128],
                        ident[s * K:(s + 1) * K, s * K:(s + 1) * K])
                    nc.vector.tensor_copy(catT[:, 4 + cb, :], tr_ps[:])
                # h = relu(cat @ W1 + b1)  (K, HR)
                h_ps = gen_tile([K, HR], F32)
                for cb in range(8):
                    nc.tensor.matmul(h_ps[:], catT[:, cb, :], w1[:, cb, :],
                                     start=(cb == 0), stop=False)
                nc.tensor.matmul(h_ps[:], ones_row[:], b1r[:],
                                 start=False, stop=True)
                h_bf = sb.tile([K, HR], BF16, tag="h_bf")
                nc.scalar.activation(h_bf[:], h_ps[:], AF.Relu)
                # hT blocks
                hT = sb.tile([128, 4, K], BF16, tag="hT")
                for cb in range(4):
                    tr_ps = gen_tile([128, K], BF16)
                    nc.tensor.transpose(tr_ps[:],
                                        h_bf[:, cb * 128:(cb + 1) * 128],
                                        ident[:K, :K])
                    nc.vector.tensor_copy(hT[:, cb, :], tr_ps[:])
                # refinement = h @ W2 + b2 ; refined += 0.1*refinement
                rf_ps = gen_tile([K, C], F32)
                for cb in range(4):
                    nc.tensor.matmul(rf_ps[:], hT[:, cb, :], w2[:, cb, :],
                                     start=(cb == 0), stop=False)
                nc.tensor.matmul(rf_ps[:], ones_row[:], b2r[:],
                                 start=False, stop=True)
                nc.vector.scalar_tensor_tensor(out=ref_f[s + 1][:],
                                               in0=rf_ps[:], scalar=0.1,
                                               in1=ref_f[s][:],
                                               op0=OP.mult, op1=OP.add)
                if s < S - 1:
                    ref_bf = sb.tile([K, C], BF16, tag="ref_bf")
                    nc.vector.tensor_copy(ref_bf[:], ref_f[s + 1][:])
            nc.sync.dma_start(out=ref_out[:], in_=ref_f[S][:])

            # ------------- confidence main loop ------------------------------
            conf_ps = [psA.tile([K, 512], F32, tag="conf_ps", name=f"conf_ps{_j}")
                       for _j in range(NCH)]
            n_mm = 2 * K  # accumulation ops per conf psum chunk
            mm_i = 0
            for k in range(K):
                for hb in range(2):
                    rt = rp.tile([128, NQS], BF16, tag="rt")
                    # relu(hq + hp[k]) : split across DVE (3/5) and ACT (2/5)
                    if (2 * k + hb) % 5 < 3:
                        nc.vector.tensor_scalar(
                            out=rt[:], in0=hqt[:, hb, :],
                            scalar1=hpbT[:, hb, k:k + 1], scalar2=0.0,
                            op0=OP.add, op1=OP.max)
                    else:
                        nc.scalar.activation(rt[:], hqt[:, hb, :], AF.Relu,
                                             bias=hpbT[:, hb, k:k + 1])
                    for j in range(NCH):
                        nc.tensor.matmul(conf_ps[j][:],
                                         w2d[:, hb, k, :],
                                         rt[:, j * 512:(j + 1) * 512],
                                         start=(mm_i == 0),
                                         stop=(mm_i == n_mm - 1))
                    mm_i += 1

            # tanh( 0.5*raw + 0.5*bc2 ), accumulate over n
            half_bc2 = sb.tile([128, 1], F32, tag="half_bc2")
            nc.vector.tensor_scalar(out=half_bc2[:], in0=bc2c[:], scalar1=0.5,
                                    scalar2=None, op0=OP.mult)
            th_scr = sb.tile([K, 512], BF16, tag="th_scr")
            tsum = sb.tile([K, NCH], F32, tag="tsum")
            for j in range(NCH):
                nc.scalar.activation(th_scr[:], conf_ps[j][:], AF.Tanh,
                                     bias=half_bc2[:K, :], scale=0.5,
                                     accum_out=tsum[:, j:j + 1])
            tsm = sb.tile([K, 1], F32, tag="tsm")
            nc.vector.tensor_reduce(tsm[:], tsum[:], axis=mybir.AxisListType.X,
                                    op=OP.add)

            # ------------- AllReduce #2 (tiny) + final conf ------------------
            ar2_in = dram.tile([K, 1], F32)
            ar2_out = dram.tile([K, 1], F32)
            nc.gpsimd.dma_start(out=ar2_in[:], in_=tsm[:])
            nc.gpsimd.collective_compute(
                "AllReduce", OP.add,
                replica_groups=[list(range(N_CORES))],
                ins=[ar2_in.opt()], outs=[ar2_out.opt()],
            )
            ar2_sb = sb.tile([K, 1], F32, tag="ar2_sb")
            nc.gpsimd.dma_start(out=ar2_sb[:], in_=ar2_out[:])
            conf_f = sb.tile([K, 1], F32, tag="conf_f")
            nc.vector.tensor_scalar(out=conf_f[:], in0=ar2_sb[:],
                                    scalar1=0.5 / NQ, scalar2=0.5,
                                    op0=OP.mult, op1=OP.add)
            nc.sync.dma_start(out=conf_out[:], in_=conf_f[:])

    nc.compile()
    return nc


def _prep_inputs(prototypes, query_features, query_distances,
                 W1, b1, W2, b2, Wc1, bc1, Wc2, bc2):
    """Host-side sharding + layout staging (no cross-tensor arithmetic)."""
    f32 = np.float32
    qf = np.asarray(query_features, f32)
    qd = np.asarray(query_distances, f32)
    W1 = np.asarray(W1, f32); b1 = np.asarray(b1, f32)
    W2 = np.asarray(W2, f32); b2 = np.asarray(b2, f32)
    Wc1 = np.asarray(Wc1, f32); bc1 = np.asarray(bc1, f32)
    Wc2 = np.asarray(Wc2, f32); bc2 = np.asarray(bc2, f32)
    pt = np.asarray(prototypes, f32)

    def blk(a, nb):  # (nb*128, m) -> (128, nb, m)
        n, m = a.shape
        return np.ascontiguousarray(
            a.reshape(nb, 128, m).transpose(1, 0, 2)).astype(BF)

    shared = {
        "w1": blk(W1, 8),
        "w2": blk(W2, 4),
        "wq": blk(Wc1[C:], 4),
        "wp": blk(Wc1[:C], 4),
        "ptT": blk(np.ascontiguousarray(pt.T), 4),
        "pt": pt,
        "b1r": b1.reshape(1, HR).astype(BF),
        "b2r": b2.reshape(1, C).astype(BF),
        "bc1r": bc1.reshape(1, HC).astype(BF),
        "bc2c": np.full((128, 1), float(bc2.reshape(-1)[0]), f32),
        "ident": np.eye(128, dtype=f32).astype(BF),
    }
    w2dm = np.zeros((128, 2, K, K), f32)
    for hb in range(2):
        for kk in range(K):
            w2dm[:, hb, kk, kk] = Wc2[hb * 128:(hb + 1) * 128, 0]
    shared["w2d"] = w2dm.astype(BF)

    in_maps = []
    for s in range(N_CORES):
        qf_sh = qf[s * NQS:(s + 1) * NQS]          # (2048, 512)
        qd_sh = qd[s * NQS:(s + 1) * NQS]          # (2048, 32)
        qfn = qf_sh.reshape(128, NB, C).astype(BF)  # n = p*16+g
        # qft[c%128, c//128, g*128+p] = qf_sh[p*16+g, c]
        qft = np.ascontiguousarray(
            qf_sh.reshape(128, NB, C).transpose(2, 1, 0)  # (C, NB, 128)
            .reshape(C, NQS).reshape(4, 128, NQS).transpose(1, 0, 2)
        ).astype(BF)
        dist = np.ascontiguousarray(qd_sh.reshape(128, NB, K))
        m = dict(shared)
        m.update({"qfn": qfn, "qft": qft, "dist": dist})
        in_maps.append(m)
    return in_maps


def kernel(**inputs):
    if "nc" not in _CACHE:
        _CACHE["nc"] = _build()
    nc = _CACHE["nc"]
    in_maps = _prep_inputs(**inputs)
    res = run_bass_kernel_spmd(nc, in_maps, core_ids=list(range(N_CORES)),
                               trace=TRACE, **TRACE_KWARGS)
    _CACHE["last_result"] = res
    refined = np.asarray(res.results[0]["refined"], np.float32).reshape(K, C)
    conf = np.asarray(res.results[0]["conf"], np.float32).reshape(K)
    return refined, conf
